# revision 23
# baseline (speedup 1.0000x reference)
"""Trainium2 Bass kernel for nn_DecoderBlock (self-attn + cross-attn + FFN, post-LN).

Sharding: 8 cores = (batch b in {0,1}) x (query block qi in {0..3} of 512 rows).
Each core computes its 512 output rows end-to-end. K/V projections are
sharded: each core projects only its own 512-position panel of K and V (for
both attentions) and the panels are exchanged with AllGathers across the
4-core replica group of the batch. A dummy 16B AllGather fires first to
absorb the one-time collective bootstrap barrier so the real AllGathers
start the moment their panels are projected. All host-side tensors and
bounce/gather buffers are partition-major so every DMA moves multi-KB
contiguous rows per partition (the DMA queues are descriptor-rate bound).

All matmuls run in bfloat16 with fp32 PSUM accumulation (fp8 DoubleRow was
tried and trips the power governor's 50% utilization cap, costing more than
it saves); K/V panels are stored fp8e3. Layernorm statistics are computed
on float32r copies so the stats matmuls stay full-rate.

Attention keeps activations transposed [d, s]: scores use KT chunks as the
stationary operand with two heads packed into the 128-row PE array via
tile_position; softmax is exp(s/8 - 4) with the normalizer produced by an
extra ones-column on V (M=65 matmul; the column travels through the
AllGather) and divided out after accumulation. The attention loop is
head-pair outer / panel inner so the AV accumulation stays in PSUM across
the whole sequence. Causal masking is a per-core 0/1 bf16 mask multiply on
the exp tiles; the cross-attention key mask is folded into the V panel
eviction (partition = key position there, so it fuses into the psum copy).
"""

import numpy as np
import ml_dtypes

import concourse.bass as bass
import concourse.mybir as mybir
import concourse.tile as tile
from concourse import bacc
from concourse.bass import ds
from concourse.bass_utils import run_bass_kernel_spmd

F32 = mybir.dt.float32
F32R = mybir.dt.float32r
BF16 = mybir.dt.bfloat16
FP8 = mybir.dt.float8e3
AF = mybir.ActivationFunctionType
ALU = mybir.AluOpType

B, S, D, H, DK, DFF = 2, 2048, 1024, 16, 64, 4096
NCORES = 8
QS = 512            # query rows per core
DC = D // 128       # 8 d-chunks
FC = DFF // 128     # 32 dff-chunks
PANEL = 512         # kpos panel size (= one core's contribution)
NPANEL = S // PANEL # 4
NSC = PANEL // 128  # 4 kpos chunks per panel
NHP = H // 2        # 8 head pairs
VW = H * (DK + 1)   # 1040: V panel row width incl per-head ones column
KROW = DC * PANEL          # 4096 B/partition: K panel bounce row
VROW = 2 * NSC * (VW // 2)  # 4160 B/partition: V panel bounce row
LN_EPS = 1e-5
EXP_BIAS = -4.0     # exp(s/8 - 4): overflow safety; cancels in the normalizer
RG = [[0, 1, 2, 3], [4, 5, 6, 7]]  # replica groups (one per batch)


def _dchunks(ap):
    """[128, (c n)] partition-major dram AP -> [128, c, n] view."""
    return ap.rearrange("p (c n) -> p c n", n=QS)


tap_layout = {}


def _build(tap=None):
    nc = bacc.Bacc("TRN2", target_bir_lowering=False, debug=False,
                   num_devices=NCORES)

    def inp(name, shape, dt=BF16):
        return nc.dram_tensor(name, shape, dt, kind="ExternalInput").ap()

    xoT = inp("xoT", [128, DC * QS])   # x[b].T rows, partition-major
    eoT = inp("eoT", [128, DC * QS])   # enc[b].T rows, partition-major
    msk = inp("msk", [128, (S // 128) * QS])  # causal mask, partition-major
    vmso = inp("vmso", [128, NSC], F32)  # src_mask for the core's own panel
    # packed projection weights: [n_mc, n_kc//4, 128, 512];
    # [mc, k4, p, jj*128+m] = W[(4*k4+jj)*128+p, mc*128+m]
    w_sa = {t: inp(f"w_sa{t}", [DC, DC // 4, 128, 512]) for t in "ko"}
    w_ca = {t: inp(f"w_ca{t}", [DC, DC // 4, 128, 512]) for t in "qo"}
    # SBUF-prefetched projection weights, partition-major [128, mc, k4, 512]
    w_saq = inp("w_saqP", [128, DC * (DC // 4) * 512])
    w_cak = inp("w_cakP", [128, DC * (DC // 4) * 512])
    # V-projection weights in moving layout [p, (kc nh m')]
    w_sav = inp("w_savP", [128, DC * 2 * 512])
    w_cav = inp("w_cavP", [128, DC * 2 * 512])
    w_ff1 = inp("w_ff1", [FC, DC // 4, 128, 512])
    w_ff2 = inp("w_ff2", [DC, FC // 4, 128, 512])
    fb1 = inp("fb1", [128, FC], F32)   # ff_b1 in [128, chunk] layout
    fb2 = inp("fb2", [128, DC], F32)
    lnb = inp("lnb", [128, 6 * DC], F32)  # g1,b1,g2,b2,g3,b3 packed
    outT = nc.dram_tensor("outT", [128, DC, QS], F32, kind="ExternalOutput").ap()
    dbg = nc.dram_tensor("dbg", [128, 40, QS], F32, kind="ExternalOutput").ap() \
        if tap else None
    tapped = []

    def tapit(name, ap):
        if tap and (tap == "all" or name in tap):
            tapped.append((name, ap))

    with tile.TileContext(nc) as tc:
        with tc.tile_pool(name="glob", bufs=1) as G, \
             tc.tile_pool(name="acts", bufs=2) as ACTS, \
             tc.tile_pool(name="dram", bufs=1, space="DRAM") as DRAM:

            # Dummy 16B AllGather fired first: absorbs the one-time
            # collective bootstrap barrier (~45-65us) + first-trigger
            # latency so the real K/V AllGathers start immediately.
            dmy = G.tile([1, 4], F32)
            nc.vector.memset(dmy[:], 0.0)
            dmyb = DRAM.tile([1, 4], F32)
            nc.scalar.dma_start(dmyb[:], dmy[:])
            dmyg = DRAM.tile([4, 4], F32)
            nc.gpsimd.collective_compute(
                "AllGather", ALU.bypass, replica_groups=RG,
                ins=[dmyb[:]], outs=[dmyg[:]])

            ones_f = G.tile([128, 64], F32)
            nc.vector.memset(ones_f[:], 1.0)
            ones = G.tile([128, 1], F32R)
            nc.vector.tensor_copy(ones[:], ones_f[:, 0:1])
            cexpb = G.tile([128, 1], F32)
            nc.vector.memset(cexpb[:], EXP_BIAS)
            cleps = G.tile([128, 1], F32)
            nc.vector.memset(cleps[:], LN_EPS)
            lnbt = G.tile([128, 6 * DC], F32)
            nc.scalar.dma_start(lnbt[:], lnb)
            fb1t = G.tile([128, FC], F32)
            nc.scalar.dma_start(fb1t[:], fb1)
            fb2t = G.tile([128, DC], F32)
            nc.scalar.dma_start(fb2t[:], fb2)
            vmst = G.tile([128, NSC], F32)
            nc.scalar.dma_start(vmst[:], vmso)

            WPOOL = None

            def proj_from_dram(wpk, rhs, evict, PSP, n_mc=DC, n_kc=DC,
                               wtag="w"):
                """psum[mc] = sum_kc w[kc,mc-chunk].T @ rhs[:,kc,:]; evict(mc, psum)."""
                for mc in range(n_mc):
                    ps = PSP.tile([128, QS], F32, tag="pj")
                    for k4 in range(n_kc // 4):
                        wt = WPOOL.tile([128, 4, 128], BF16, tag=wtag)
                        nc.sync.dma_start(
                            wt[:], wpk[mc, k4].rearrange("p (j m) -> p j m", j=4))
                        for j in range(4):
                            kc = 4 * k4 + j
                            nc.tensor.matmul(ps[:], wt[:, j, :], rhs[:, kc, :],
                                             start=(kc == 0), stop=(kc == n_kc - 1))
                    evict(mc, ps)

            def proj_from_sbuf(wt, rhs, evict, PSP, n_mc=DC, n_kc=DC):
                """Same as proj_from_dram but with SBUF-resident packed weights
                wt [128, n_mc, n_kc//4, 512]."""
                for mc in range(n_mc):
                    ps = PSP.tile([128, QS], F32, tag="pj")
                    for kc in range(n_kc):
                        k4, j = kc // 4, kc % 4
                        nc.tensor.matmul(ps[:], wt[:, mc, k4, ds(j * 128, 128)],
                                         rhs[:, kc, :],
                                         start=(kc == 0), stop=(kc == n_kc - 1))
                    evict(mc, ps)

            def layernorm(xpre, gcol, bcol, out, TMP, PSP):
                """out[:,mc,:] = (xpre - mu)/sd * g + b, stats over d (partition+chunks).

                xpre must be f32r so the stats matmuls run full-rate."""
                pmu = PSP.tile([1, QS], F32, tag="pj")
                for kc in range(DC):
                    nc.tensor.matmul(pmu[:], ones[:], xpre[:, kc, :],
                                     start=(kc == 0), stop=(kc == DC - 1))
                pm2 = PSP.tile([1, QS], F32, tag="pj")
                for kc in range(DC):
                    sq = TMP.tile([128, QS], F32R, tag="sq")
                    nc.scalar.activation(sq[:], xpre[:, kc, :], AF.Square)
                    nc.tensor.matmul(pm2[:], ones[:], sq[:],
                                     start=(kc == 0), stop=(kc == DC - 1))
                st = TMP.tile([1, 5, QS], F32, tag="st")
                mu = st[0:1, 0, :]
                ex2 = st[0:1, 1, :]
                var = st[0:1, 2, :]
                sd = st[0:1, 3, :]
                rstd = st[0:1, 4, :]
                nc.vector.tensor_scalar_mul(mu, pmu[:], 1.0 / D)
                nc.vector.tensor_scalar_mul(ex2, pm2[:], 1.0 / D)
                nc.vector.tensor_tensor(var, mu, mu, ALU.mult)
                nc.vector.tensor_sub(var, ex2, var)
                nc.scalar.activation(sd, var, AF.Sqrt, bias=cleps[0:1, :])
                nc.vector.reciprocal(rstd, sd)
                mub = TMP.tile([128, QS], F32, tag="mub")
                nc.gpsimd.partition_broadcast(mub[:], mu)
                rsb = TMP.tile([128, QS], F32, tag="rsb")
                nc.gpsimd.partition_broadcast(rsb[:], rstd)
                for mc in range(DC):
                    t = TMP.tile([128, QS], F32, tag="t")
                    nc.vector.tensor_sub(t[:], xpre[:, mc, :], mub[:])
                    nc.vector.tensor_mul(t[:], t[:], rsb[:])
                    nc.vector.tensor_scalar(
                        out=out[:, mc, :], in0=t[:],
                        scalar1=gcol[:, mc:mc + 1], scalar2=bcol[:, mc:mc + 1],
                        op0=ALU.mult, op1=ALU.add)

            def kv_panel_proj(src, wk, wv, kb, vb, PSK, PV, apply_vms=False):
                """Project own K/V panel from src, write bounce tensors.

                wk: SBUF prefetch tile (tuple) or packed dram tensor for the
                K weights; wv: dram [128, (kc nh m')] V weights in moving
                layout, streamed per chunk. Bounce layouts are
                partition-major so the write and the post-AllGather loads
                are contiguous-4KB DMAs. When apply_vms, the own-panel
                src_mask is folded into the V psum eviction (partition =
                key position there)."""
                kt_own = KVP.tile([128, DC, PANEL], FP8, tag="kt_own")

                def evk(mc, psum):
                    nc.vector.tensor_copy(kt_own[:, mc, :], psum[:])
                if isinstance(wk, tuple):
                    proj_from_sbuf(wk[0], src, evk, PSK)
                else:
                    proj_from_dram(wk, src, evk, PSK, wtag="wk")
                nc.sync.dma_start(
                    kb.rearrange("p (c n) -> p c n", n=PANEL), kt_own[:])

                vo = KVP.tile([128, 2, NSC, VW // 2], FP8, tag="vo")
                nc.vector.memset(
                    vo.rearrange("p a s (h e) -> p a s h e", e=DK + 1)
                    [:, :, :, :, DK], 1.0)
                if apply_vms:
                    # mask the per-head ones column (the normalizer must not
                    # count masked keys); partition = key position
                    for sc in range(NSC):
                        nc.vector.tensor_scalar_mul(
                            vo.rearrange("p a s (h e) -> p a s h e", e=DK + 1)
                            [:, :, sc, :, DK],
                            vo.rearrange("p a s (h e) -> p a s h e", e=DK + 1)
                            [:, :, sc, :, DK],
                            vmst[:, sc:sc + 1])
                for nh in range(2):
                    pss = [PV.tile([128, QS], F32, tag=f"pv{i}", name=f"vps{nh}{i}")
                           for i in range(NSC)]
                    for kc in range(DC):
                        wvtile = WPOOL.tile([128, PANEL], BF16, tag="wv")
                        nc.sync.dma_start(
                            wvtile[:], wv[:, ds(kc * 1024 + nh * 512, 512)])
                        for sc in range(NSC):
                            nc.tensor.matmul(
                                pss[sc][:], src[:, kc, ds(sc * 128, 128)],
                                wvtile[:],
                                start=(kc == 0), stop=(kc == DC - 1))
                    for sc in range(NSC):
                        dst = vo[:, nh, sc, :].rearrange(
                            "p (h e) -> p h e", e=DK + 1)[:, :, 0:DK]
                        srcp = pss[sc][:].rearrange("p (h d) -> p h d", d=DK)
                        if apply_vms:
                            nc.vector.tensor_scalar_mul(
                                dst, srcp, vmst[:, sc:sc + 1])
                        else:
                            nc.vector.tensor_copy(dst, srcp)
                nc.sync.dma_start(
                    vb.rearrange("p (a s c) -> p a s c", a=2, s=NSC), vo[:])

            def ag(bounce, shape):
                g = DRAM.tile(shape, FP8)
                nc.gpsimd.collective_compute(
                    "AllGather", ALU.bypass, replica_groups=RG,
                    ins=[bounce[:]], outs=[g[:]])
                return g

            def load_panels(KT, v1, ksrc, vsrc, eng):
                """ksrc/vsrc: panel index -> [128, KROW/VROW] dram AP."""
                for p in range(NPANEL):
                    eng.dma_start(
                        KT[:, p, :, :],
                        ksrc(p).rearrange("pp (c n) -> pp c n", n=PANEL))
                for p in range(NPANEL):
                    eng.dma_start(
                        v1[:, p],
                        vsrc(p).rearrange(
                            "pp (dh sc hh e) -> pp dh sc hh e",
                            dh=2, sc=NSC, hh=8))

            def attention_inner(QT, KT, v1, mt, ON, masked):
                with tc.tile_pool(name="pp", bufs=2) as PP, \
                     tc.tile_pool(name="rnbp", bufs=1) as RNB, \
                     tc.tile_pool(name="pso", bufs=2, space="PSUM") as PSO, \
                     tc.tile_pool(name="pss", bufs=2, space="PSUM") as PSS:
                    for hp in range(NHP):
                        dh, hh = hp // 4, (2 * hp) % 8
                        po0 = PSO.tile([65, QS], F32, tag="po0")
                        po1 = PSO.tile([65, QS], F32, tag="po1")
                        for p in range(NPANEL):
                            for sc in range(NSC):
                                ci = p * NSC + sc
                                ps = PSS.tile([128, 2, QS], F32, tag="ps")
                                nc.tensor.matmul(
                                    ps[:, 0, :],
                                    KT[0:64, p, hp, ds(sc * 128, 128)],
                                    QT[0:64, hp, :], start=True, stop=True)
                                nc.tensor.matmul(
                                    ps[:, 1, :],
                                    KT[64:128, p, hp, ds(sc * 128, 128)],
                                    QT[64:128, hp, :], start=True, stop=True,
                                    tile_position=(64, 0))
                                p01 = PP.tile([128, 2, QS], BF16, tag="p01")
                                nc.scalar.activation(p01[:], ps[:], AF.Exp,
                                                     scale=0.125, bias=cexpb[:])
                                if masked:
                                    nc.vector.tensor_mul(
                                        p01[:, 0, :], p01[:, 0, :], mt[:, ci, :])
                                    nc.vector.tensor_mul(
                                        p01[:, 1, :], p01[:, 1, :], mt[:, ci, :])
                                nc.tensor.matmul(
                                    po0[:], v1[:, p, dh, sc, hh, :],
                                    p01[:, 0, :], start=(ci == 0),
                                    stop=(ci == NPANEL * NSC - 1))
                                nc.tensor.matmul(
                                    po1[:], v1[:, p, dh, sc, hh + 1, :],
                                    p01[:, 1, :], start=(ci == 0),
                                    stop=(ci == NPANEL * NSC - 1))
                        # normalize: ON[:, hp] = po / po[64] (per head)
                        nrm = RNB.tile([1, 2, QS], F32, tag="nrm")
                        nc.vector.tensor_copy(nrm[0:1, 0, :], po0[64:65, :])
                        nc.vector.tensor_copy(nrm[0:1, 1, :], po1[64:65, :])
                        rr = RNB.tile([1, 2, QS], F32, tag="rr")
                        nc.vector.reciprocal_approx_fast(rr[:], nrm[:])
                        rnb = RNB.tile([64, 2, QS], F32, tag="rnb")
                        nc.gpsimd.partition_broadcast(rnb[:, 0, :], rr[0:1, 0, :])
                        nc.gpsimd.partition_broadcast(rnb[:, 1, :], rr[0:1, 1, :])
                        nc.vector.tensor_mul(ON[0:64, hp, :],
                                             po0[0:64, :], rnb[:, 0, :])
                        nc.vector.tensor_mul(ON[64:128, hp, :],
                                             po1[0:64, :], rnb[:, 1, :])

            def attn_epilogue(w_o, ON, res, gcol, bcol, sfx):
                nonlocal WPOOL
                with tc.tile_pool(name="aepi", bufs=1) as E, \
                     tc.tile_pool(name="tmp", bufs=2) as TMP, \
                     tc.tile_pool(name="pse", bufs=2, space="PSUM") as PSE, \
                     tc.tile_pool(name="wo", bufs=6) as WPOOL:
                    xpre = E.tile([128, DC, QS], F32R)

                    def evo(mc, ps):
                        nc.vector.tensor_add(xpre[:, mc, :], ps[:], res[:, mc, :])
                    proj_from_dram(w_o, ON, evo, PSE)
                    tapit("xpre" + sfx, xpre)
                    xnext = ACTS.tile([128, DC, QS], BF16, tag="act")
                    layernorm(xpre, gcol, bcol, xnext, TMP, PSE)
                    tapit("xn" + sfx, xnext)
                return xnext

            # ---- load own-panel activations ----
            xo = ACTS.tile([128, DC, QS], BF16, tag="act")
            nc.sync.dma_start(xo[:], _dchunks(xoT))

            g1, b1 = lnbt[:, 0:DC], lnbt[:, DC:2 * DC]
            g2, b2 = lnbt[:, 2 * DC:3 * DC], lnbt[:, 3 * DC:4 * DC]
            g3, b3 = lnbt[:, 4 * DC:5 * DC], lnbt[:, 5 * DC:6 * DC]

            kb_sa = DRAM.tile([128, KROW], FP8)
            vb_sa = DRAM.tile([128, VROW], FP8)
            kvb_ca = DRAM.tile([128, KROW + VROW], FP8)

            with tc.tile_pool(name="attn_ca", bufs=1) as A_CA:
                with tc.tile_pool(name="attn_sa", bufs=1) as A_SA:
                    QT_sa = A_SA.tile([128, DC, QS], BF16)
                    KT_sa = A_SA.tile([128, NPANEL, DC, PANEL], FP8)
                    v1_sa = A_SA.tile([128, NPANEL, 2, NSC, 8, DK + 1], FP8)
                    mt = A_SA.tile([128, S // 128, QS], BF16)
                    ON_sa = A_SA.tile([128, DC, QS], BF16)
                    KT_ca = A_CA.tile([128, NPANEL, DC, PANEL], FP8)
                    v1_ca = A_CA.tile([128, NPANEL, 2, NSC, 8, DK + 1], FP8)

                    # ---- K/V panel projections + AllGathers ----
                    with tc.tile_pool(name="pref", bufs=1) as PRE, \
                         tc.tile_pool(name="kvp", bufs=1) as KVP, \
                         tc.tile_pool(name="wkv", bufs=6) as WPOOL, \
                         tc.tile_pool(name="psk", bufs=2, space="PSUM") as PSK, \
                         tc.tile_pool(name="pv", bufs=1, space="PSUM") as PV:
                        # prefetches ride the scalar hwdge queue; the sync
                        # queue carries the streamed K weights + bounces
                        eo = KVP.tile([128, DC, QS], BF16, tag="eo")
                        nc.scalar.dma_start(eo[:], _dchunks(eoT))
                        wqs = PRE.tile([128, DC, DC // 4, 512], BF16)
                        nc.scalar.dma_start(
                            wqs[:], w_saq.rearrange(
                                "p (a b c) -> p a b c", a=DC, b=DC // 4))
                        wck = PRE.tile([128, DC, DC // 4, 512], BF16)
                        nc.scalar.dma_start(
                            wck[:], w_cak.rearrange(
                                "p (a b c) -> p a b c", a=DC, b=DC // 4))
                        nc.scalar.dma_start(
                            mt[:], msk.rearrange("p (c q) -> p c q", q=QS))

                        kv_panel_proj(xo, w_sa["k"], w_sav, kb_sa, vb_sa,
                                      PSK, PV)
                        kg_sa = ag(kb_sa, [NPANEL * 128, KROW])
                        vg_sa = ag(vb_sa, [NPANEL * 128, VROW])

                        # SA panel loads on the scalar queue, issued right
                        # after the AGs so they fire the moment data lands
                        load_panels(KT_sa, v1_sa,
                                    lambda p: kg_sa[ds(128 * p, 128), :],
                                    lambda p: vg_sa[ds(128 * p, 128), :],
                                    nc.scalar)

                        # SA Q projection from prefetched weights (overlaps
                        # the SA collectives)
                        def evq(mc, ps):
                            nc.vector.tensor_copy(QT_sa[:, mc, :], ps[:])
                        proj_from_sbuf(wqs, xo, evq, PSK)

                        kv_panel_proj(eo, (wck,), w_cav,
                                      kvb_ca[:, 0:KROW],
                                      kvb_ca[:, KROW:KROW + VROW],
                                      PSK, PV, apply_vms=True)
                        kvg_ca = ag(kvb_ca, [NPANEL * 128, KROW + VROW])
                    tapit("QTsa", QT_sa)

                    # CA panels load during SA attention (gpsimd queue,
                    # naturally ordered after the CA collective)
                    load_panels(KT_ca, v1_ca,
                                lambda p: kvg_ca[ds(128 * p, 128), 0:KROW],
                                lambda p: kvg_ca[ds(128 * p, 128),
                                                 KROW:KROW + VROW],
                                nc.gpsimd)

                    attention_inner(QT_sa, KT_sa, v1_sa, mt, ON_sa,
                                    masked=True)
                    tapit("ONsa", ON_sa)
                    x1 = attn_epilogue(w_sa["o"], ON_sa, xo, g1, b1, "sa")

                # ---- CA attention ----
                QT_ca = A_CA.tile([128, DC, QS], BF16)
                with tc.tile_pool(name="wq", bufs=6) as WPOOL, \
                     tc.tile_pool(name="psq", bufs=2, space="PSUM") as PSQ:
                    def evq2(mc, ps):
                        nc.vector.tensor_copy(QT_ca[:, mc, :], ps[:])
                    proj_from_dram(w_ca["q"], x1, evq2, PSQ)
                tapit("QTca", QT_ca)
                ON_ca = A_CA.tile([128, DC, QS], BF16)
                attention_inner(QT_ca, KT_ca, v1_ca, None, ON_ca,
                                masked=False)
                tapit("ONca", ON_ca)
                x2 = attn_epilogue(w_ca["o"], ON_ca, x1, g2, b2, "ca")

            # ---- FFN ----
            with tc.tile_pool(name="ffn", bufs=1) as F, \
                 tc.tile_pool(name="tmp2", bufs=2) as TMP, \
                 tc.tile_pool(name="psf", bufs=2, space="PSUM") as PSF, \
                 tc.tile_pool(name="wf", bufs=6) as WPOOL:
                h1 = F.tile([128, FC, QS], BF16)

                def ev1(fc, ps):
                    nc.scalar.activation(h1[:, fc, :], ps[:], AF.Relu,
                                         bias=fb1t[:, fc:fc + 1])
                proj_from_dram(w_ff1, x2, ev1, PSF, n_mc=FC, n_kc=DC)

                tapit("h1a", h1[:, 0:8, :])
                xpre = F.tile([128, DC, QS], F32R)

                def ev2(mc, ps):
                    nc.vector.scalar_tensor_tensor(
                        out=xpre[:, mc, :], in0=ps[:],
                        scalar=fb2t[:, mc:mc + 1], in1=x2[:, mc, :],
                        op0=ALU.add, op1=ALU.add)
                proj_from_dram(w_ff2, h1, ev2, PSF, n_mc=DC, n_kc=FC)

                tapit("xpreff", xpre)
                out = F.tile([128, DC, QS], F32)
                layernorm(xpre, g3, b3, out, TMP, PSF)
                tapit("outf", out)
                for mc in range(DC):
                    nc.sync.dma_start(outT[:, mc, :], out[:, mc, :])
            if tap:
                base = 0
                tap_layout.clear()
                for name, t in tapped:
                    sh = t.shape
                    nparts = sh[0]
                    assert len(sh) == 3 and sh[2] == QS
                    tap_layout[name] = (base, sh[1], nparts)
                    for cci in range(sh[1]):
                        nc.sync.dma_start(
                            dbg[0:nparts, base + cci, :].bitcast(t.dtype),
                            t[:, cci, :])
                    base += sh[1]
                assert base <= 40

    nc.compile()
    return nc


_NC_CACHE = None


def _get_nc():
    global _NC_CACHE
    if _NC_CACHE is None:
        _NC_CACHE = _build()
    return _NC_CACHE


BF16NP = ml_dtypes.bfloat16


def _pack_w(w):
    """[K, M] fp32 -> packed bf16 [M//128, K//512, 128, 512] (see _build)."""
    K, M = w.shape
    nk4, nmc = K // 512, M // 128
    p = w.reshape(nk4, 4, 128, nmc, 128).transpose(3, 0, 2, 1, 4)
    return np.ascontiguousarray(p.reshape(nmc, nk4, 128, 512)
                                .astype(BF16NP))


def _pack_wP(w):
    """[K, M] fp32 -> partition-major prefetch bf16 [128, mc*k4*512].

    [p, mc, k4, jj*128+m] = W[(4*k4+jj)*128+p, mc*128+m]."""
    K, M = w.shape
    nk4, nmc = K // 512, M // 128
    p = w.reshape(nk4, 4, 128, nmc, 128).transpose(2, 3, 0, 1, 4)
    return np.ascontiguousarray(p.reshape(128, nmc * nk4 * 512)
                                .astype(BF16NP))


def _pack_wv(w):
    """[D, M] fp32 -> V-moving layout bf16 [128, (D//128)*2*(M//2)].

    [p, kc, nh, m'] = w[kc*128 + p, nh*(M//2) + m']."""
    K, M = w.shape
    p = w.reshape(K // 128, 128, 2, M // 2).transpose(1, 0, 2, 3)
    return np.ascontiguousarray(p.reshape(128, -1).astype(BF16NP))


def _pm(a, dt):
    """[D, N] -> partition-major [128, (D//128)*N]."""
    Dd, N = a.shape
    return np.ascontiguousarray(
        a.reshape(Dd // 128, 128, N).transpose(1, 0, 2).reshape(128, -1)
        .astype(dt))


def _prep_in_maps(x, enc, tgt_mask, src_mask,
                  sa_wq, sa_wk, sa_wv, sa_wo,
                  ca_wq, ca_wk, ca_wv, ca_wo,
                  ff_w1, ff_b1, ff_w2, ff_b2,
                  ln1_g, ln1_b, ln2_g, ln2_b, ln3_g, ln3_b):
    f32 = np.float32

    def c(a):
        return np.ascontiguousarray(np.asarray(a), dtype=f32)

    xT = [np.asarray(x, dtype=f32)[b].T for b in range(B)]
    eT = [np.asarray(enc, dtype=f32)[b].T for b in range(B)]
    tm = np.asarray(tgt_mask)[0, 0].astype(f32).T            # [k, q]
    sm = np.asarray(src_mask)[0, 0, 0].astype(f32)           # [k]

    def percol(v, nchunks):
        return c(np.asarray(v).reshape(nchunks, 128).T)

    lnb = c(np.concatenate(
        [percol(v, DC) for v in [ln1_g, ln1_b, ln2_g, ln2_b, ln3_g, ln3_b]],
        axis=1))
    fb1 = percol(ff_b1, FC)
    fb2 = percol(ff_b2, DC)
    shared = {
        "lnb": lnb, "fb1": fb1, "fb2": fb2,
        "w_sak": _pack_w(c(sa_wk)), "w_sao": _pack_w(c(sa_wo)),
        "w_caq": _pack_w(c(ca_wq)), "w_cao": _pack_w(c(ca_wo)),
        "w_saqP": _pack_wP(c(sa_wq)), "w_cakP": _pack_wP(c(ca_wk)),
        "w_savP": _pack_wv(c(sa_wv)), "w_cavP": _pack_wv(c(ca_wv)),
        "w_ff1": _pack_w(c(ff_w1)), "w_ff2": _pack_w(c(ff_w2)),
    }
    in_maps = []
    for core in range(NCORES):
        b, qi = core // 4, core % 4
        q0 = qi * QS
        m = dict(shared)
        m["xoT"] = _pm(xT[b][:, q0:q0 + QS], BF16NP)
        m["eoT"] = _pm(eT[b][:, q0:q0 + QS], BF16NP)
        m["msk"] = _pm(tm[:, q0:q0 + QS], BF16NP)
        # src_mask over the core's own key panel, [128, NSC] (key = sc*128+p)
        m["vmso"] = c(sm[q0:q0 + QS].reshape(NSC, 128).T)
        in_maps.append(m)
    return in_maps


def _gather_out(res):
    out = np.empty((B, S, D), dtype=np.float32)
    for core in range(NCORES):
        b, qi = core // 4, core % 4
        q0 = qi * QS
        arr = res.results[core]["outT"]  # [128, DC, QS]
        out[b, q0:q0 + QS, :] = arr.transpose(1, 0, 2).reshape(D, QS).T
    return out


def kernel(**inputs):
    in_maps = _prep_in_maps(**inputs)
    nc = _get_nc()
    res = run_bass_kernel_spmd(nc, in_maps, core_ids=list(range(NCORES)))
    return _gather_out(res)


def _profiled_run(inputs):
    """Test-only: run with NTFF tracing to get HW exec time."""
    in_maps = _prep_in_maps(**inputs)
    nc = _get_nc()
    return run_bass_kernel_spmd(nc, in_maps, core_ids=list(range(NCORES)),
                                trace=True)


# revision 29
# speedup vs baseline: 1.1209x; 1.1209x over previous
"""Trainium2 Bass kernel for nn_DecoderBlock (self-attn + cross-attn + FFN, post-LN).

Sharding: 8 cores = (batch b in {0,1}) x (query block qi in {0..3} of 512 rows).
Each core computes its 512 output rows end-to-end. K/V projections are
sharded: each core projects only its own 512-position panel of K and V (for
both attentions) and the panels are exchanged with AllGathers across the
4-core replica group of the batch. A dummy 16B AllGather fires first to
absorb the one-time collective bootstrap barrier so the real AllGathers
start the moment their panels are projected. All host-side tensors and
bounce/gather buffers are partition-major so every DMA moves multi-KB
contiguous rows per partition (the DMA queues are descriptor-rate bound).

All matmuls run in bfloat16 with fp32 PSUM accumulation (fp8 DoubleRow was
tried and trips the power governor's 50% utilization cap, costing more than
it saves); K/V panels are stored fp8e3. Layernorm statistics are computed
on float32r copies so the stats matmuls stay full-rate.

Attention keeps activations transposed [d, s]: scores use KT chunks as the
stationary operand with two heads packed into the 128-row PE array via
tile_position; softmax is exp(s/8 - 4) with the normalizer produced by an
extra ones-column on V (M=65 matmul; the column travels through the
AllGather) and divided out after accumulation. The attention loop is
head-pair outer / panel inner so the AV accumulation stays in PSUM across
the whole sequence. Causal masking is a per-core 0/1 bf16 mask multiply on
the exp tiles; the cross-attention key mask is folded into the V panel
eviction (partition = key position there, so it fuses into the psum copy).
"""

import numpy as np
import ml_dtypes

import concourse.bass as bass
import concourse.mybir as mybir
import concourse.tile as tile
from concourse import bacc
from concourse.bass import ds
from concourse.bass_utils import run_bass_kernel_spmd

F32 = mybir.dt.float32
F32R = mybir.dt.float32r
BF16 = mybir.dt.bfloat16
FP8 = mybir.dt.float8e3
F8 = mybir.dt.float8e3
AF = mybir.ActivationFunctionType
ALU = mybir.AluOpType

B, S, D, H, DK, DFF = 2, 2048, 1024, 16, 64, 4096
NCORES = 8
QS = 512            # query rows per core
DC = D // 128       # 8 d-chunks
FC = DFF // 128     # 32 dff-chunks
PANEL = 512         # kpos panel size (= one core's contribution)
NPANEL = S // PANEL # 4
NSC = PANEL // 128  # 4 kpos chunks per panel
NHP = H // 2        # 8 head pairs
VW = H * (DK + 1)   # 1040: V panel row width incl per-head ones column
KROW = DC * PANEL          # 4096 B/partition: K panel bounce row
VROW = 2 * NSC * (VW // 2)  # 4160 B/partition: V panel bounce row
LN_EPS = 1e-5
WS = 64.0           # host-side fp8 weight scale; divided out at psum evict
WSI = 1.0 / WS
EXP_BIAS = -4.0     # exp(s/8 - 4): overflow safety; cancels in the normalizer
RG = [[0, 1, 2, 3], [4, 5, 6, 7]]  # replica groups (one per batch)


def _dchunks(ap):
    """[128, (c n)] partition-major dram AP -> [128, c, n] view."""
    return ap.rearrange("p (c n) -> p c n", n=QS)


tap_layout = {}


def _build(tap=None):
    nc = bacc.Bacc("TRN2", target_bir_lowering=False, debug=False,
                   num_devices=NCORES)

    def inp(name, shape, dt=BF16):
        return nc.dram_tensor(name, shape, dt, kind="ExternalInput").ap()

    xoT = inp("xoT", [128, DC * QS])   # x[b].T rows, partition-major
    eoT = inp("eoT", [128, DC * QS])   # enc[b].T rows, partition-major
    msk = inp("msk", [128, (S // 128) * QS])  # causal mask, partition-major
    vmso = inp("vmso", [128, NSC], F32)  # src_mask for the core's own panel
    # fp8e3 stationary projection weights: [mc, p, (kc m)];
    # [mc, p, kc*128+m] = W[kc*128+p, mc*128+m]  (1KB/partition rows)
    w_sak = inp("w_sak8", [DC, 128, DC * 128], F8)
    w_saq = inp("w_saq8", [DC, 128, DC * 128], F8)
    w_sao = inp("w_sao8", [DC, 128, DC * 128], F8)
    w_caq = inp("w_caq8", [DC, 128, DC * 128], F8)
    w_cao = inp("w_cao8", [DC, 128, DC * 128], F8)
    # SBUF-prefetched CA-K weights, partition-major [128, (mc kc m)]
    w_cak = inp("w_cak8", [128, DC * DC * 128], F8)
    # fp8e3 V-projection weights in moving layout [p, (kc nh m')]
    w_sav = inp("w_sav8", [128, DC * 2 * 512], F8)
    w_cav = inp("w_cav8", [128, DC * 2 * 512], F8)
    w_ff1 = inp("w_ff1", [FC, DC // 4, 128, 512])
    w_ff2 = inp("w_ff2", [DC, FC // 4, 128, 512])
    fb1 = inp("fb1", [128, FC], F32)   # ff_b1 in [128, chunk] layout
    fb2 = inp("fb2", [128, DC], F32)
    lnb = inp("lnb", [128, 6 * DC], F32)  # g1,b1,g2,b2,g3,b3 packed
    outT = nc.dram_tensor("outT", [128, DC, QS], F32, kind="ExternalOutput").ap()
    dbg = nc.dram_tensor("dbg", [128, 40, QS], F32, kind="ExternalOutput").ap() \
        if tap else None
    tapped = []

    def tapit(name, ap):
        if tap and (tap == "all" or name in tap):
            tapped.append((name, ap))

    with tile.TileContext(nc) as tc:
        with tc.tile_pool(name="glob", bufs=1) as G, \
             tc.tile_pool(name="acts", bufs=2) as ACTS, \
             tc.tile_pool(name="wglob", bufs=6) as WPOOL, \
             tc.tile_pool(name="dram", bufs=1, space="DRAM") as DRAM:

            # Dummy 16B AllGather fired first: absorbs the one-time
            # collective bootstrap barrier (~45-65us) + first-trigger
            # latency so the real K/V AllGathers start immediately.
            dmy = G.tile([1, 4], F32)
            nc.vector.memset(dmy[:], 0.0)
            dmyb = DRAM.tile([1, 4], F32)
            nc.scalar.dma_start(dmyb[:], dmy[:])
            dmyg = DRAM.tile([4, 4], F32)
            nc.gpsimd.collective_compute(
                "AllGather", ALU.bypass, replica_groups=RG,
                ins=[dmyb[:]], outs=[dmyg[:]])

            ones_f = G.tile([128, 64], F32)
            nc.vector.memset(ones_f[:], 1.0)
            ones = G.tile([128, 1], F32R)
            nc.vector.tensor_copy(ones[:], ones_f[:, 0:1])
            cexpb = G.tile([128, 1], F32)
            nc.vector.memset(cexpb[:], EXP_BIAS)
            cleps = G.tile([128, 1], F32)
            nc.vector.memset(cleps[:], LN_EPS)
            lnbt = G.tile([128, 6 * DC], F32)
            nc.scalar.dma_start(lnbt[:], lnb)
            fb1t = G.tile([128, FC], F32)
            nc.scalar.dma_start(fb1t[:], fb1)
            fb2t = G.tile([128, DC], F32)
            nc.scalar.dma_start(fb2t[:], fb2)
            vmst = G.tile([128, NSC], F32)
            nc.scalar.dma_start(vmst[:], vmso)

            def proj_from_dram(wpk, rhs, evict, PSP, n_mc=DC, n_kc=DC,
                               wtag="w", alt_queue=False):
                """psum[mc] = sum_kc w[kc,mc-chunk].T @ rhs[:,kc,:]; evict(mc, psum).

                alt_queue: stream weights alternately on sync/scalar so a
                DMA-heavy phase (FFN) is not bound by one queue."""
                for mc in range(n_mc):
                    ps = PSP.tile([128, QS], F32, tag="pj")
                    for k4 in range(n_kc // 4):
                        wt = WPOOL.tile([128, 4, 128], BF16, tag=wtag)
                        eng = nc.scalar if (alt_queue and (mc + k4) % 2) \
                            else nc.sync
                        eng.dma_start(
                            wt[:], wpk[mc, k4].rearrange("p (j m) -> p j m", j=4))
                        for j in range(4):
                            kc = 4 * k4 + j
                            nc.tensor.matmul(ps[:], wt[:, j, :], rhs[:, kc, :],
                                             start=(kc == 0), stop=(kc == n_kc - 1))
                    evict(mc, ps)

            def proj_f8_dram(wpk8, rhs, evict, PSP, n_mc=DC, n_kc=DC):
                """fp8e3-stationary projection, one 1KB-row DMA per mc."""
                for mc in range(n_mc):
                    wt = WPOOL.tile([128, n_kc, 128], F8, tag="w8")
                    nc.sync.dma_start(
                        wt[:], wpk8[mc].rearrange("p (c m) -> p c m", m=128))
                    ps = PSP.tile([128, QS], F32, tag="pj")
                    for kc in range(n_kc):
                        nc.tensor.matmul(ps[:], wt[:, kc, :], rhs[:, kc, :],
                                         start=(kc == 0), stop=(kc == n_kc - 1))
                    evict(mc, ps)

            def proj_f8_sbuf(wt, rhs, evict, PSP, n_mc=DC, n_kc=DC):
                """fp8e3-stationary projection from SBUF-resident weights
                wt [128, n_mc, n_kc, 128]."""
                for mc in range(n_mc):
                    ps = PSP.tile([128, QS], F32, tag="pj")
                    for kc in range(n_kc):
                        nc.tensor.matmul(ps[:], wt[:, mc, kc, :], rhs[:, kc, :],
                                         start=(kc == 0), stop=(kc == n_kc - 1))
                    evict(mc, ps)

            def layernorm(xpre, gcol, bcol, out, TMP, PSP):
                """out[:,mc,:] = (xpre - mu)/sd * g + b, stats over d (partition+chunks).

                xpre must be f32r so the stats matmuls run full-rate."""
                pmu = PSP.tile([1, QS], F32, tag="pj")
                for kc in range(DC):
                    nc.tensor.matmul(pmu[:], ones[:], xpre[:, kc, :],
                                     start=(kc == 0), stop=(kc == DC - 1))
                pm2 = PSP.tile([1, QS], F32, tag="pj")
                for kc in range(DC):
                    sq = TMP.tile([128, QS], F32R, tag="sq")
                    nc.scalar.activation(sq[:], xpre[:, kc, :], AF.Square)
                    nc.tensor.matmul(pm2[:], ones[:], sq[:],
                                     start=(kc == 0), stop=(kc == DC - 1))
                st = TMP.tile([1, 5, QS], F32, tag="st")
                mu = st[0:1, 0, :]
                ex2 = st[0:1, 1, :]
                var = st[0:1, 2, :]
                sd = st[0:1, 3, :]
                rstd = st[0:1, 4, :]
                nc.vector.tensor_scalar_mul(mu, pmu[:], 1.0 / D)
                nc.vector.tensor_scalar_mul(ex2, pm2[:], 1.0 / D)
                nc.vector.tensor_tensor(var, mu, mu, ALU.mult)
                nc.vector.tensor_sub(var, ex2, var)
                nc.scalar.activation(sd, var, AF.Sqrt, bias=cleps[0:1, :])
                nc.vector.reciprocal(rstd, sd)
                mub = TMP.tile([128, QS], F32, tag="mub")
                nc.gpsimd.partition_broadcast(mub[:], mu)
                rsb = TMP.tile([128, QS], F32, tag="rsb")
                nc.gpsimd.partition_broadcast(rsb[:], rstd)
                for mc in range(DC):
                    t = TMP.tile([128, QS], F32, tag="t")
                    nc.vector.tensor_sub(t[:], xpre[:, mc, :], mub[:])
                    nc.vector.tensor_mul(t[:], t[:], rsb[:])
                    nc.vector.tensor_scalar(
                        out=out[:, mc, :], in0=t[:],
                        scalar1=gcol[:, mc:mc + 1], scalar2=bcol[:, mc:mc + 1],
                        op0=ALU.mult, op1=ALU.add)

            def kv_panel_proj(src, wk, wv, kb, vb, PSK, PV, apply_vms=False):
                """Project own K/V panel from src, write bounce tensors.

                wk: SBUF prefetch tile (tuple) or packed dram tensor for the
                K weights; wv: dram [128, (kc nh m')] V weights in moving
                layout, streamed per chunk. Bounce layouts are
                partition-major so the write and the post-AllGather loads
                are contiguous-4KB DMAs. When apply_vms, the own-panel
                src_mask is folded into the V psum eviction (partition =
                key position there)."""
                kt_own = KVP.tile([128, DC, PANEL], FP8, tag="kt_own")

                def evk(mc, psum):
                    nc.vector.tensor_scalar_mul(kt_own[:, mc, :], psum[:], WSI)
                if isinstance(wk, tuple):
                    proj_f8_sbuf(wk[0], src, evk, PSK)
                else:
                    proj_f8_dram(wk, src, evk, PSK)
                nc.sync.dma_start(
                    kb.rearrange("p (c n) -> p c n", n=PANEL), kt_own[:])

                vo = KVP.tile([128, 2, NSC, VW // 2], FP8, tag="vo")
                nc.vector.memset(
                    vo.rearrange("p a s (h e) -> p a s h e", e=DK + 1)
                    [:, :, :, :, DK], 1.0)
                if apply_vms:
                    # mask the per-head ones column (the normalizer must not
                    # count masked keys); partition = key position
                    for sc in range(NSC):
                        nc.vector.tensor_scalar_mul(
                            vo.rearrange("p a s (h e) -> p a s h e", e=DK + 1)
                            [:, :, sc, :, DK],
                            vo.rearrange("p a s (h e) -> p a s h e", e=DK + 1)
                            [:, :, sc, :, DK],
                            vmst[:, sc:sc + 1])
                for nh in range(2):
                    pss = [PV.tile([128, QS], F32, tag=f"pv{i}", name=f"vps{nh}{i}")
                           for i in range(NSC)]
                    for kc in range(DC):
                        wvtile = WPOOL.tile([128, PANEL], F8, tag="wv")
                        nc.sync.dma_start(
                            wvtile[:], wv[:, ds(kc * 1024 + nh * 512, 512)])
                        for sc in range(NSC):
                            nc.tensor.matmul(
                                pss[sc][:], src[:, kc, ds(sc * 128, 128)],
                                wvtile[:],
                                start=(kc == 0), stop=(kc == DC - 1))
                    for sc in range(NSC):
                        dst = vo[:, nh, sc, :].rearrange(
                            "p (h e) -> p h e", e=DK + 1)[:, :, 0:DK]
                        srcp = pss[sc][:].rearrange("p (h d) -> p h d", d=DK)
                        if apply_vms:
                            nc.vector.tensor_scalar(
                                out=dst, in0=srcp,
                                scalar1=vmst[:, sc:sc + 1], scalar2=WSI,
                                op0=ALU.mult, op1=ALU.mult)
                        else:
                            nc.vector.tensor_scalar_mul(dst, srcp, WSI)
                nc.sync.dma_start(
                    vb.rearrange("p (a s c) -> p a s c", a=2, s=NSC), vo[:])

            def ag(bounce, shape):
                g = DRAM.tile(shape, FP8)
                nc.gpsimd.collective_compute(
                    "AllGather", ALU.bypass, replica_groups=RG,
                    ins=[bounce[:]], outs=[g[:]])
                return g

            def load_panels(KT, v1, ksrc, vsrc, eng):
                """ksrc/vsrc: panel index -> [128, KROW/VROW] dram AP."""
                for p in range(NPANEL):
                    eng.dma_start(
                        KT[:, p, :, :],
                        ksrc(p).rearrange("pp (c n) -> pp c n", n=PANEL))
                for p in range(NPANEL):
                    eng.dma_start(
                        v1[:, p],
                        vsrc(p).rearrange(
                            "pp (dh sc hh e) -> pp dh sc hh e",
                            dh=2, sc=NSC, hh=8))

            def attention_inner(QT, KT, v1, mt, ON, masked):
                with tc.tile_pool(name="pp", bufs=2) as PP, \
                     tc.tile_pool(name="rnbp", bufs=1) as RNB, \
                     tc.tile_pool(name="pso", bufs=2, space="PSUM") as PSO, \
                     tc.tile_pool(name="pss", bufs=2, space="PSUM") as PSS:
                    for hp in range(NHP):
                        dh, hh = hp // 4, (2 * hp) % 8
                        po0 = PSO.tile([65, QS], F32, tag="po0")
                        po1 = PSO.tile([65, QS], F32, tag="po1")
                        for p in range(NPANEL):
                            for sc in range(NSC):
                                ci = p * NSC + sc
                                ps = PSS.tile([128, 2, QS], F32, tag="ps")
                                nc.tensor.matmul(
                                    ps[:, 0, :],
                                    KT[0:64, p, hp, ds(sc * 128, 128)],
                                    QT[0:64, hp, :], start=True, stop=True)
                                nc.tensor.matmul(
                                    ps[:, 1, :],
                                    KT[64:128, p, hp, ds(sc * 128, 128)],
                                    QT[64:128, hp, :], start=True, stop=True,
                                    tile_position=(64, 0))
                                p01 = PP.tile([128, 2, QS], BF16, tag="p01")
                                nc.scalar.activation(p01[:], ps[:], AF.Exp,
                                                     scale=0.125, bias=cexpb[:])
                                if masked:
                                    nc.vector.tensor_mul(
                                        p01[:, 0, :], p01[:, 0, :], mt[:, ci, :])
                                    nc.vector.tensor_mul(
                                        p01[:, 1, :], p01[:, 1, :], mt[:, ci, :])
                                nc.tensor.matmul(
                                    po0[:], v1[:, p, dh, sc, hh, :],
                                    p01[:, 0, :], start=(ci == 0),
                                    stop=(ci == NPANEL * NSC - 1))
                                nc.tensor.matmul(
                                    po1[:], v1[:, p, dh, sc, hh + 1, :],
                                    p01[:, 1, :], start=(ci == 0),
                                    stop=(ci == NPANEL * NSC - 1))
                        # normalize: ON[:, hp] = po / po[64] (per head)
                        nrm = RNB.tile([1, 2, QS], F32, tag="nrm")
                        nc.vector.tensor_copy(nrm[0:1, 0, :], po0[64:65, :])
                        nc.vector.tensor_copy(nrm[0:1, 1, :], po1[64:65, :])
                        rr = RNB.tile([1, 2, QS], F32, tag="rr")
                        nc.vector.reciprocal_approx_fast(rr[:], nrm[:])
                        rnb = RNB.tile([64, 2, QS], F32, tag="rnb")
                        nc.gpsimd.partition_broadcast(rnb[:, 0, :], rr[0:1, 0, :])
                        nc.gpsimd.partition_broadcast(rnb[:, 1, :], rr[0:1, 1, :])
                        nc.vector.tensor_mul(ON[0:64, hp, :],
                                             po0[0:64, :], rnb[:, 0, :])
                        nc.vector.tensor_mul(ON[64:128, hp, :],
                                             po1[0:64, :], rnb[:, 1, :])

            def attn_epilogue(w_o, ON, res, gcol, bcol, sfx):
                with tc.tile_pool(name="aepi", bufs=1) as E, \
                     tc.tile_pool(name="tmp", bufs=2) as TMP, \
                     tc.tile_pool(name="pse", bufs=2, space="PSUM") as PSE:
                    xpre = E.tile([128, DC, QS], F32R)

                    def evo(mc, ps):
                        nc.vector.scalar_tensor_tensor(
                            out=xpre[:, mc, :], in0=ps[:], scalar=WSI,
                            in1=res[:, mc, :], op0=ALU.mult, op1=ALU.add)
                    proj_f8_dram(w_o, ON, evo, PSE)
                    tapit("xpre" + sfx, xpre)
                    xnext = ACTS.tile([128, DC, QS], BF16, tag="act")
                    layernorm(xpre, gcol, bcol, xnext, TMP, PSE)
                    tapit("xn" + sfx, xnext)
                return xnext

            # ---- load own-panel activations ----
            xo = ACTS.tile([128, DC, QS], BF16, tag="act")
            nc.sync.dma_start(xo[:], _dchunks(xoT))

            g1, b1 = lnbt[:, 0:DC], lnbt[:, DC:2 * DC]
            g2, b2 = lnbt[:, 2 * DC:3 * DC], lnbt[:, 3 * DC:4 * DC]
            g3, b3 = lnbt[:, 4 * DC:5 * DC], lnbt[:, 5 * DC:6 * DC]

            kb_sa = DRAM.tile([128, KROW], FP8)
            vb_sa = DRAM.tile([128, VROW], FP8)
            kvb_ca = DRAM.tile([128, KROW + VROW], FP8)

            with tc.tile_pool(name="attn_ca", bufs=1) as A_CA:
                with tc.tile_pool(name="attn_sa", bufs=1) as A_SA:
                    QT_sa = A_SA.tile([128, DC, QS], BF16)
                    KT_sa = A_SA.tile([128, NPANEL, DC, PANEL], FP8)
                    v1_sa = A_SA.tile([128, NPANEL, 2, NSC, 8, DK + 1], FP8)
                    mt = A_SA.tile([128, S // 128, QS], BF16)
                    ON_sa = A_SA.tile([128, DC, QS], BF16)
                    KT_ca = A_CA.tile([128, NPANEL, DC, PANEL], FP8)
                    v1_ca = A_CA.tile([128, NPANEL, 2, NSC, 8, DK + 1], FP8)

                    # ---- K/V panel projections + AllGathers ----
                    with tc.tile_pool(name="pref", bufs=1) as PRE, \
                         tc.tile_pool(name="kvp", bufs=1) as KVP, \
                         tc.tile_pool(name="psk", bufs=2, space="PSUM") as PSK, \
                         tc.tile_pool(name="pv", bufs=1, space="PSUM") as PV:
                        # prefetches ride the scalar hwdge queue; the sync
                        # queue carries the streamed K/V/Q weights + bounces
                        eo = KVP.tile([128, DC, QS], BF16, tag="eo")
                        nc.scalar.dma_start(eo[:], _dchunks(eoT))
                        wck = PRE.tile([128, DC, DC, 128], F8)
                        nc.scalar.dma_start(
                            wck[:], w_cak.rearrange(
                                "p (a c m) -> p a c m", a=DC, c=DC))
                        nc.scalar.dma_start(
                            mt[:], msk.rearrange("p (c q) -> p c q", q=QS))

                        kv_panel_proj(xo, w_sak, w_sav, kb_sa, vb_sa,
                                      PSK, PV)
                        kg_sa = ag(kb_sa, [NPANEL * 128, KROW])
                        vg_sa = ag(vb_sa, [NPANEL * 128, VROW])

                        # SA panel loads on the scalar queue, issued right
                        # after the AGs so they fire the moment data lands
                        load_panels(KT_sa, v1_sa,
                                    lambda p: kg_sa[ds(128 * p, 128), :],
                                    lambda p: vg_sa[ds(128 * p, 128), :],
                                    nc.scalar)

                        # SA Q projection, streamed (overlaps SA collectives)
                        def evq(mc, ps):
                            nc.vector.tensor_scalar_mul(QT_sa[:, mc, :], ps[:],
                                                        WSI)
                        proj_f8_dram(w_saq, xo, evq, PSK)

                        kv_panel_proj(eo, (wck,), w_cav,
                                      kvb_ca[:, 0:KROW],
                                      kvb_ca[:, KROW:KROW + VROW],
                                      PSK, PV, apply_vms=True)
                        kvg_ca = ag(kvb_ca, [NPANEL * 128, KROW + VROW])
                    tapit("QTsa", QT_sa)

                    # CA panels load during SA attention (gpsimd queue,
                    # naturally ordered after the CA collective)
                    load_panels(KT_ca, v1_ca,
                                lambda p: kvg_ca[ds(128 * p, 128), 0:KROW],
                                lambda p: kvg_ca[ds(128 * p, 128),
                                                 KROW:KROW + VROW],
                                nc.gpsimd)

                    attention_inner(QT_sa, KT_sa, v1_sa, mt, ON_sa,
                                    masked=True)
                    tapit("ONsa", ON_sa)
                    x1 = attn_epilogue(w_sao, ON_sa, xo, g1, b1, "sa")

                # ---- CA attention ----
                QT_ca = A_CA.tile([128, DC, QS], BF16)
                with tc.tile_pool(name="psq", bufs=2, space="PSUM") as PSQ:
                    def evq2(mc, ps):
                        nc.vector.tensor_scalar_mul(QT_ca[:, mc, :], ps[:], WSI)
                    proj_f8_dram(w_caq, x1, evq2, PSQ)
                tapit("QTca", QT_ca)
                ON_ca = A_CA.tile([128, DC, QS], BF16)
                attention_inner(QT_ca, KT_ca, v1_ca, None, ON_ca,
                                masked=False)
                tapit("ONca", ON_ca)
                x2 = attn_epilogue(w_cao, ON_ca, x1, g2, b2, "ca")

            # ---- FFN ----
            with tc.tile_pool(name="ffn", bufs=1) as F, \
                 tc.tile_pool(name="tmp2", bufs=2) as TMP, \
                 tc.tile_pool(name="psf", bufs=2, space="PSUM") as PSF:
                h1 = F.tile([128, FC, QS], BF16)

                def ev1(fc, ps):
                    nc.scalar.activation(h1[:, fc, :], ps[:], AF.Relu,
                                         bias=fb1t[:, fc:fc + 1])
                proj_from_dram(w_ff1, x2, ev1, PSF, n_mc=FC, n_kc=DC,
                               alt_queue=True)

                tapit("h1a", h1[:, 0:8, :])
                xpre = F.tile([128, DC, QS], F32R)

                def ev2(mc, ps):
                    nc.vector.scalar_tensor_tensor(
                        out=xpre[:, mc, :], in0=ps[:],
                        scalar=fb2t[:, mc:mc + 1], in1=x2[:, mc, :],
                        op0=ALU.add, op1=ALU.add)
                proj_from_dram(w_ff2, h1, ev2, PSF, n_mc=DC, n_kc=FC,
                               alt_queue=True)

                tapit("xpreff", xpre)
                out = F.tile([128, DC, QS], F32)
                layernorm(xpre, g3, b3, out, TMP, PSF)
                tapit("outf", out)
                for mc in range(DC):
                    eng = nc.scalar if mc % 2 else nc.sync
                    eng.dma_start(outT[:, mc, :], out[:, mc, :])
            if tap:
                base = 0
                tap_layout.clear()
                for name, t in tapped:
                    sh = t.shape
                    nparts = sh[0]
                    assert len(sh) == 3 and sh[2] == QS
                    tap_layout[name] = (base, sh[1], nparts)
                    for cci in range(sh[1]):
                        nc.sync.dma_start(
                            dbg[0:nparts, base + cci, :].bitcast(t.dtype),
                            t[:, cci, :])
                    base += sh[1]
                assert base <= 40

    nc.compile()
    return nc


_NC_CACHE = None


def _get_nc():
    global _NC_CACHE
    if _NC_CACHE is None:
        _NC_CACHE = _build()
    return _NC_CACHE


BF16NP = ml_dtypes.bfloat16


def _pack_w(w):
    """[K, M] fp32 -> packed bf16 [M//128, K//512, 128, 512] (see _build)."""
    K, M = w.shape
    nk4, nmc = K // 512, M // 128
    p = w.reshape(nk4, 4, 128, nmc, 128).transpose(3, 0, 2, 1, 4)
    return np.ascontiguousarray(p.reshape(nmc, nk4, 128, 512)
                                .astype(BF16NP))


F8NP = ml_dtypes.float8_e3m4


def _pack_w8(w):
    """[K, M] fp32 -> fp8e3 stationary [M//128, 128, (K//128)*128].

    [mc, p, kc*128+m] = W[kc*128+p, mc*128+m]  (1KB/partition rows)."""
    K, M = w.shape
    p = (w * 64.0).reshape(K // 128, 128, M // 128, 128).transpose(2, 1, 0, 3)
    return np.ascontiguousarray(p.reshape(M // 128, 128, -1).astype(F8NP))


def _pack_w8P(w):
    """[K, M] fp32 -> fp8e3 prefetch [128, (M//128)*(K//128)*128].

    [p, mc, kc, m] = W[kc*128+p, mc*128+m]."""
    K, M = w.shape
    p = (w * 64.0).reshape(K // 128, 128, M // 128, 128).transpose(1, 2, 0, 3)
    return np.ascontiguousarray(p.reshape(128, -1).astype(F8NP))


def _pack_wv8(w):
    """[D, M] fp32 -> V-moving layout fp8e3 [128, (D//128)*2*(M//2)].

    [p, kc, nh, m'] = w[kc*128 + p, nh*(M//2) + m']."""
    K, M = w.shape
    p = (w * 64.0).reshape(K // 128, 128, 2, M // 2).transpose(1, 0, 2, 3)
    return np.ascontiguousarray(p.reshape(128, -1).astype(F8NP))


def _pm(a, dt):
    """[D, N] -> partition-major [128, (D//128)*N]."""
    Dd, N = a.shape
    return np.ascontiguousarray(
        a.reshape(Dd // 128, 128, N).transpose(1, 0, 2).reshape(128, -1)
        .astype(dt))


def _prep_in_maps(x, enc, tgt_mask, src_mask,
                  sa_wq, sa_wk, sa_wv, sa_wo,
                  ca_wq, ca_wk, ca_wv, ca_wo,
                  ff_w1, ff_b1, ff_w2, ff_b2,
                  ln1_g, ln1_b, ln2_g, ln2_b, ln3_g, ln3_b):
    f32 = np.float32

    def c(a):
        return np.ascontiguousarray(np.asarray(a), dtype=f32)

    xT = [np.asarray(x, dtype=f32)[b].T for b in range(B)]
    eT = [np.asarray(enc, dtype=f32)[b].T for b in range(B)]
    tm = np.asarray(tgt_mask)[0, 0].astype(f32).T            # [k, q]
    sm = np.asarray(src_mask)[0, 0, 0].astype(f32)           # [k]

    def percol(v, nchunks):
        return c(np.asarray(v).reshape(nchunks, 128).T)

    lnb = c(np.concatenate(
        [percol(v, DC) for v in [ln1_g, ln1_b, ln2_g, ln2_b, ln3_g, ln3_b]],
        axis=1))
    fb1 = percol(ff_b1, FC)
    fb2 = percol(ff_b2, DC)
    shared = {
        "lnb": lnb, "fb1": fb1, "fb2": fb2,
        "w_sak8": _pack_w8(c(sa_wk)), "w_sao8": _pack_w8(c(sa_wo)),
        "w_saq8": _pack_w8(c(sa_wq)), "w_caq8": _pack_w8(c(ca_wq)),
        "w_cao8": _pack_w8(c(ca_wo)), "w_cak8": _pack_w8P(c(ca_wk)),
        "w_sav8": _pack_wv8(c(sa_wv)), "w_cav8": _pack_wv8(c(ca_wv)),
        "w_ff1": _pack_w(c(ff_w1)), "w_ff2": _pack_w(c(ff_w2)),
    }
    in_maps = []
    for core in range(NCORES):
        b, qi = core // 4, core % 4
        q0 = qi * QS
        m = dict(shared)
        m["xoT"] = _pm(xT[b][:, q0:q0 + QS], BF16NP)
        m["eoT"] = _pm(eT[b][:, q0:q0 + QS], BF16NP)
        m["msk"] = _pm(tm[:, q0:q0 + QS], BF16NP)
        # src_mask over the core's own key panel, [128, NSC] (key = sc*128+p)
        m["vmso"] = c(sm[q0:q0 + QS].reshape(NSC, 128).T)
        in_maps.append(m)
    return in_maps


def _gather_out(res):
    out = np.empty((B, S, D), dtype=np.float32)
    for core in range(NCORES):
        b, qi = core // 4, core % 4
        q0 = qi * QS
        arr = res.results[core]["outT"]  # [128, DC, QS]
        out[b, q0:q0 + QS, :] = arr.transpose(1, 0, 2).reshape(D, QS).T
    return out


def kernel(**inputs):
    in_maps = _prep_in_maps(**inputs)
    nc = _get_nc()
    res = run_bass_kernel_spmd(nc, in_maps, core_ids=list(range(NCORES)))
    return _gather_out(res)


def _profiled_run(inputs):
    """Test-only: run with NTFF tracing to get HW exec time."""
    in_maps = _prep_in_maps(**inputs)
    nc = _get_nc()
    return run_bass_kernel_spmd(nc, in_maps, core_ids=list(range(NCORES)),
                                trace=True)
